# revision 29
# baseline (speedup 1.0000x reference)
"""Causal single-head attention (B=4, S=2048, D=1024) on 8 trn2 NeuronCores.

Sharding: core = (batch b, parity h).  Each core owns the 1024 queries of
batch b in 256-row blocks {2t+h : t=0..3} (interleaved for causal load
balance) and computes its own K/V projections for the full sequence.

On-chip dataflow (per core, SPMD-uniform):
  phase 0: QT[e,q]  = Wq^T x_q^T        (bf16 matmuls, fp32 psum)
  phase 1: KT[e,k]  = Wk^T x^T,  V[k,e] = x Wv   (4 key chunks of 512)
  phase 2: per slot t (q-block 2t+h, 256 queries):
       scoresT[k,q] = KT^T QT   over k-tiles 0..4t+3   (keys 0..512(t+1))
       expT = exp(scoresT/32) * mask   (multiplicative 0/1 causal mask on
                                        the last 4 k-tiles of the slot)
       denom[1,q] += ones^T expT       (sum over keys via matmul)
       outT[e,q]  += V^T expT          (PV accumulate in psum)
       out[q,e] = transpose(outT) * (1/denom)   (PE transpose + DVE scale)
"""

import os
import sys
from collections import deque
from contextlib import ExitStack

import numpy as np
import ml_dtypes

import concourse.bass as bass
import concourse.mybir as mybir
import concourse.tile as tile
from concourse import bacc
from concourse import bass_utils
from concourse.masks import make_identity

B, S, D = 4, 2048, 1024
P = 128
QB = 256          # queries per slot
NSLOT = 4         # slots per core
NQ = QB * NSLOT   # queries per core
NCORES = 8
F32 = mybir.dt.float32
BF16 = mybir.dt.bfloat16
SCALE = 1.0 / 32.0  # 1/sqrt(D)


def _build_kernel():
    nc = bacc.Bacc("TRN2", target_bir_lowering=False, debug=False,
                   num_devices=NCORES)

    xt4 = nc.dram_tensor("xt4", [8, P, 8, 256], F32, kind="ExternalInput").ap()
    xq4 = nc.dram_tensor("xq4", [4, P, 8, 256], F32, kind="ExternalInput").ap()
    wq4 = nc.dram_tensor("wq4", [4, P, 8, 256], F32, kind="ExternalInput").ap()
    wk4 = nc.dram_tensor("wk4", [4, P, 8, 256], F32, kind="ExternalInput").ap()
    wv4 = nc.dram_tensor("wv4", [4, P, 8, 256], F32, kind="ExternalInput").ap()
    maskT = nc.dram_tensor("maskT", [P, 4, 256], BF16, kind="ExternalInput").ap()
    out = nc.dram_tensor("out", [NQ, D], F32, kind="ExternalOutput").ap()
    # cores 2b (h=0) and 2b+1 (h=1) of batch b exchange K/V halves
    GROUPS = [[0, 1], [2, 3], [4, 5], [6, 7]]

    with tile.TileContext(nc) as tc, ExitStack() as ctx:
        const = ctx.enter_context(tc.tile_pool(name="const", bufs=1))
        persist = ctx.enter_context(tc.tile_pool(name="persist", bufs=1))

        ident = const.tile([P, P], F32)
        make_identity(nc, ident[:])
        ones = const.tile([P, 1], BF16)
        nc.gpsimd.memset(ones[:], 1.0)
        mask_sb = const.tile([P, 4, 256], BF16)
        nc.sync.dma_start(mask_sb[:], maskT[:])

        QT = persist.tile([P, 8, NQ], BF16)    # [e_in_tile, e_tile, q]
        KT = persist.tile([P, 8, S], BF16)     # [e_in_tile, e_tile, k]
        V = persist.tile([P, 16, D], BF16)     # [k_in_tile, k_tile, e]
        denT = persist.tile([P, 2 * NSLOT], F32)
        rinv = persist.tile([P, 2 * NSLOT], F32)

        # ---------------- projection phases ----------------
        # K (and Q) are projected in full on every core — keeping scores
        # off the collective critical path — while V is projected only for
        # the core's own four 256-row query blocks {2t+h} (so the xq tiles
        # double as the V stationary operand) and pair-wise all-gathered in
        # two stages.  Gather output is rank-ordered: stage s, rank r,
        # piece j holds global block 4s+2j+r on both cores — SPMD-uniform.
        # V-own runs first so the collectives launch early; their latency
        # hides under the K/Q projections.
        with tc.tile_pool(name="wsb", bufs=1) as wsb_pool, \
             tc.tile_pool(name="stage", bufs=2) as stage_pool, \
             tc.tile_pool(name="xc", bufs=6) as xc_pool, \
             tc.tile_pool(name="kvh", bufs=1) as kvh_pool, \
             tc.tile_pool(name="ccdram", bufs=2, space="DRAM") as ccdram, \
             tc.tile_pool(name="pproj", bufs=4, space="PSUM") as pproj:

            _ld_ctr = [0]

            def load_cast(dst, dram_chunk):
                # alternate the DMA between two issue queues for bandwidth
                st = stage_pool.tile([P, 8, 256], F32, tag="stage")
                eng = nc.sync if _ld_ctr[0] % 2 == 0 else nc.scalar
                _ld_ctr[0] += 1
                eng.dma_start(st[:], dram_chunk)
                nc.vector.tensor_copy(dst, st[:])

            wq_sb = wsb_pool.tile([P, 8, D], BF16, tag="wq")
            wk_sb = wsb_pool.tile([P, 8, D], BF16, tag="wk")
            wv_sb = wsb_pool.tile([P, 8, D], BF16, tag="wv")

            def load_chunk_pair(dram4, i):
                xc = xc_pool.tile([P, 8, 512], BF16, tag="xc")
                for s2 in range(2):
                    load_cast(xc[:, :, 256 * s2:256 * (s2 + 1)],
                              dram4[2 * i + s2])
                return xc

            # Loads are emitted lazily, right before the compute group that
            # consumes them, so the DVE cast/evac FIFO order matches true
            # data-arrival order (no head-of-line blocking).

            def emit_v(s, xc):
                # V for the core's own query blocks of stage s (its blocks
                # 4s+h and 4s+2+h, i.e. xq chunk s) + pair all-gather
                vh = kvh_pool.tile([P, 4, D], BF16, tag="vh")
                for ec in range(2):
                    for k4 in range(4):
                        ps = pproj.tile([P, 512], F32, tag="pp")
                        for dt in range(8):
                            nc.tensor.matmul(
                                ps[:], xc[:, dt, P * k4:P * (k4 + 1)],
                                wv_sb[:, dt, 512 * ec:512 * (ec + 1)],
                                start=(dt == 0), stop=(dt == 7))
                        nc.vector.tensor_copy(vh[:, k4, 512 * ec:512 * (ec + 1)],
                                              ps[:])
                cc_in = ccdram.tile([P, 4096], BF16, tag="cc_in")
                cc_out = ccdram.tile([2, P, 4096], BF16, tag="cc_out")
                # all collective-adjacent DMAs live on the gpsimd queue so
                # they never head-of-line-block the input-load sync queue
                nc.gpsimd.dma_start(cc_in[:],
                                    vh[:].rearrange("p a b -> p (a b)"))
                nc.gpsimd.collective_compute(
                    "AllGather", mybir.AluOpType.bypass,
                    replica_groups=GROUPS,
                    ins=[cc_in[:]], outs=[cc_out[:]])
                # stage s, rank r, piece j -> global 256-row block 4s+2j+r
                for r in range(2):
                    co = cc_out[r].rearrange("p (a b) -> p a b", a=4)
                    for j in range(2):
                        g = 4 * s + 2 * j + r
                        nc.gpsimd.dma_start(V[:, 2 * g:2 * g + 2, :],
                                            co[:, 2 * j:2 * j + 2, :])

            def emit_k(kc, xc):
                for et in range(8):
                    ps = pproj.tile([P, 512], F32, tag="pp")
                    for dt in range(8):
                        nc.tensor.matmul(
                            ps[:], wk_sb[:, dt, P * et:P * (et + 1)],
                            xc[:, dt, :],
                            start=(dt == 0), stop=(dt == 7))
                    nc.vector.tensor_copy(KT[:, et, 512 * kc:512 * (kc + 1)],
                                          ps[:])

            def emit_q(qc, xqc):
                for et in range(8):
                    ps = pproj.tile([P, 512], F32, tag="pp")
                    for dt in range(8):
                        nc.tensor.matmul(
                            ps[:], wq_sb[:, dt, P * et:P * (et + 1)],
                            xqc[:, dt, :],
                            start=(dt == 0), stop=(dt == 7))
                    nc.vector.tensor_copy(QT[:, et, 512 * qc:512 * (qc + 1)],
                                          ps[:])

            # PE order matches DMA arrival order; loads emitted lazily
            for c in range(4):
                load_cast(wv_sb[:, :, 256 * c:256 * (c + 1)], wv4[c])
            xq_tiles = [load_chunk_pair(xq4, 0)]
            emit_v(0, xq_tiles[0])
            xq_tiles.append(load_chunk_pair(xq4, 1))
            emit_v(1, xq_tiles[1])
            for c in range(4):
                load_cast(wq_sb[:, :, 256 * c:256 * (c + 1)], wq4[c])
            emit_q(0, xq_tiles[0])
            emit_q(1, xq_tiles[1])
            for c in range(4):
                load_cast(wk_sb[:, :, 256 * c:256 * (c + 1)], wk4[c])
            xg_tiles = [load_chunk_pair(xt4, 0)]
            emit_k(0, xg_tiles[0])
            xg_tiles.append(load_chunk_pair(xt4, 1))
            emit_k(1, xg_tiles[1])
            xg_tiles.append(load_chunk_pair(xt4, 2))
            emit_k(2, xg_tiles[2])
            xg_tiles.append(load_chunk_pair(xt4, 3))
            emit_k(3, xg_tiles[3])

        # ---------------- attention phase ----------------
        with tc.tile_pool(name="ps_s", bufs=2, space="PSUM") as ps_s, \
             tc.tile_pool(name="ps_d", bufs=1, space="PSUM") as ps_d, \
             tc.tile_pool(name="ps_o", bufs=1, space="PSUM") as ps_o, \
             tc.tile_pool(name="ps_t", bufs=1, space="PSUM") as ps_t, \
             tc.tile_pool(name="expp", bufs=4) as expp, \
             tc.tile_pool(name="tmpp", bufs=2) as tmpp, \
             tc.tile_pool(name="obuf", bufs=2) as obufp, \
             tc.tile_pool(name="osb", bufs=4) as osbp, \
             tc.tile_pool(name="dendram", bufs=1, space="DRAM") as dendramp, \
             tc.tile_pool(name="dsb", bufs=2) as dsbp:

            den_dram = dendramp.tile([NSLOT, QB], F32)
            post_queue = deque()

            for t in range(NSLOT):
                nkt = 4 * (t + 1)
                po = ps_o.tile([P, 8, 256], F32, tag="po")
                pd = ps_d.tile([P, 256], F32, tag="pd")
                exp_tiles = {}

                def emit_pv(k, t=t, nkt=nkt, po=po, pd=pd, exp_tiles=None):
                    e = exp_tiles[k]
                    nc.tensor.matmul(pd[0:1, :], ones[:, 0:1], e[:],
                                     start=(k == 0), stop=(k == nkt - 1))
                    # po packs two 256-wide accumulation regions per 2KB psum
                    # bank; only one accumulation group may exist per bank, so
                    # start/stop fire on the first/last matmul touching the
                    # bank (even/odd e-tile respectively).
                    for et in range(8):
                        nc.tensor.matmul(po[:, et, :],
                                         V[:, k, P * et:P * (et + 1)], e[:],
                                         start=(k == 0 and et % 2 == 0),
                                         stop=(k == nkt - 1 and et % 2 == 1))

                for kt in range(nkt):
                    ps = ps_s.tile([P, 256], F32, tag="ps")
                    for et in range(8):
                        nc.tensor.matmul(
                            ps[:], KT[:, et, P * kt:P * (kt + 1)],
                            QT[:, et, QB * t:QB * (t + 1)],
                            start=(et == 0), stop=(et == 7))
                    e = expp.tile([P, 256], BF16, tag="exp")
                    j = kt - 4 * t
                    if j >= 0:
                        tmp = tmpp.tile([P, 256], BF16, tag="tmp")
                        nc.scalar.activation(tmp[:], ps[:],
                                             mybir.ActivationFunctionType.Exp,
                                             scale=SCALE)
                        nc.vector.tensor_tensor(e[:], tmp[:], mask_sb[:, j, :],
                                                mybir.AluOpType.mult)
                    else:
                        nc.scalar.activation(e[:], ps[:],
                                             mybir.ActivationFunctionType.Exp,
                                             scale=SCALE)
                    exp_tiles[kt] = e
                    if kt >= 1:
                        emit_pv(kt - 1, exp_tiles=exp_tiles)
                    if post_queue:
                        post_queue.popleft()()
                emit_pv(nkt - 1, exp_tiles=exp_tiles)

                # denominator -> [q,1] layout via DRAM roundtrip
                dsb = dsbp.tile([1, 256], F32, tag="den")
                nc.vector.tensor_copy(dsb[:], pd[0:1, :])
                nc.sync.dma_start(den_dram[t:t + 1, :], dsb[:])
                nc.sync.dma_start(
                    denT[:, 2 * t:2 * t + 2],
                    den_dram[t:t + 1, :].rearrange("o (c p) -> (o p) c", p=P))
                nc.vector.reciprocal(rinv[:, 2 * t:2 * t + 2],
                                     denT[:, 2 * t:2 * t + 2])

                # evacuate PV accumulator, then queue transpose+normalize work
                ob = obufp.tile([P, 8, 256], F32, tag="ob")
                nc.vector.tensor_copy(ob[:], po[:])

                for half in range(2):
                    osb = osbp.tile([P, D], F32, tag="osb")

                    def mk_tr(et, ob=ob, osb=osb, t=t, half=half):
                        def doit():
                            tr = ps_t.tile([P, P], F32, tag="tr")
                            nc.tensor.transpose(
                                tr[:], ob[:, et, P * half:P * (half + 1)],
                                ident[:])
                            # normalize on the (lightly-loaded) scalar engine
                            nc.scalar.mul(
                                osb[:, P * et:P * (et + 1)], tr[:],
                                rinv[:, 2 * t + half:2 * t + half + 1])
                        return doit

                    for et in range(8):
                        post_queue.append(mk_tr(et))

                    def mk_out(osb=osb, t=t, half=half):
                        def doit():
                            r0 = QB * t + P * half
                            nc.sync.dma_start(out[r0:r0 + P, :], osb[:])
                        return doit

                    post_queue.append(mk_out())

            while post_queue:
                post_queue.popleft()()

    nc.compile()
    return nc


_NC_CACHE = None


def _get_nc():
    global _NC_CACHE
    if _NC_CACHE is None:
        _NC_CACHE = _build_kernel()
    return _NC_CACHE


def _make_masks():
    kk = np.arange(P)[:, None]
    qq = np.arange(256)[None, :]
    diag0 = (qq >= kk).astype(np.float32)
    diag1 = (qq >= kk + P).astype(np.float32)
    m = {}
    for h in range(2):
        mt = np.zeros((P, 4, 256), np.float32)
        if h == 0:
            mt[:, 0], mt[:, 1] = diag0, diag1
        else:
            mt[:, 0], mt[:, 1] = 1.0, 1.0
            mt[:, 2], mt[:, 3] = diag0, diag1
        m[h] = mt.astype(ml_dtypes.bfloat16)
    return m


def _prep_inputs(x, Wq, Wk, Wv):
    def w4(W):
        return np.ascontiguousarray(
            W.reshape(8, P, 4, 256).transpose(2, 1, 0, 3))

    wq4, wk4, wv4 = w4(Wq), w4(Wk), w4(Wv)
    masks = _make_masks()
    in_maps = []
    for core in range(NCORES):
        b, h = divmod(core, 2)
        xb = np.asarray(x[b])
        xt4 = np.ascontiguousarray(
            xb.reshape(8, 256, 8, P).transpose(0, 3, 2, 1))
        order = np.concatenate(
            [np.arange(QB * (2 * t + h), QB * (2 * t + h) + QB)
             for t in range(NSLOT)])
        xq = xb[order]
        xq4 = np.ascontiguousarray(
            xq.reshape(4, 256, 8, P).transpose(0, 3, 2, 1))
        in_maps.append({
            "xt4": xt4, "xq4": xq4,
            "wq4": wq4, "wk4": wk4, "wv4": wv4,
            "maskT": masks[h],
        })
    return in_maps


def run(inputs, trace=False):
    nc = _get_nc()
    in_maps = _prep_inputs(inputs["x"], inputs["Wq"], inputs["Wk"],
                           inputs["Wv"])
    res = bass_utils.run_bass_kernel_spmd(
        nc, in_maps, core_ids=list(range(NCORES)), trace=trace)
    out = np.empty((B, S, D), np.float32)
    for core in range(NCORES):
        b, h = divmod(core, 2)
        oc = res.results[core]["out"]
        for t in range(NSLOT):
            out[b, QB * (2 * t + h):QB * (2 * t + h) + QB] = \
                oc[QB * t:QB * t + QB]
    return out, res


def kernel(**inputs):
    out, _ = run(inputs, trace=False)
    return out


# revision 33
# speedup vs baseline: 1.0872x; 1.0872x over previous
"""Causal single-head attention (B=4, S=2048, D=1024) on 8 trn2 NeuronCores.

Sharding: core = (batch b, parity h).  Each core owns the 1024 queries of
batch b in 256-row blocks {2t+h : t=0..3} (interleaved for causal load
balance) and computes its own K/V projections for the full sequence.

On-chip dataflow (per core, SPMD-uniform):
  phase 0: QT[e,q]  = Wq^T x_q^T        (bf16 matmuls, fp32 psum)
  phase 1: KT[e,k]  = Wk^T x^T,  V[k,e] = x Wv   (4 key chunks of 512)
  phase 2: per slot t (q-block 2t+h, 256 queries):
       scoresT[k,q] = KT^T QT   over k-tiles 0..4t+3   (keys 0..512(t+1))
       expT = exp(scoresT/32) * mask   (multiplicative 0/1 causal mask on
                                        the last 4 k-tiles of the slot)
       denom[1,q] += ones^T expT       (sum over keys via matmul)
       outT[e,q]  += V^T expT          (PV accumulate in psum)
       out[q,e] = transpose(outT) * (1/denom)   (PE transpose + DVE scale)
"""

import os
import sys
from collections import deque
from contextlib import ExitStack

import numpy as np
import ml_dtypes

import concourse.bass as bass
import concourse.mybir as mybir
import concourse.tile as tile
from concourse import bacc
from concourse import bass_utils
from concourse.masks import make_identity

B, S, D = 4, 2048, 1024
P = 128
QB = 256          # queries per slot
NSLOT = 4         # slots per core
NQ = QB * NSLOT   # queries per core
NCORES = 8
F32 = mybir.dt.float32
BF16 = mybir.dt.bfloat16
SCALE = 1.0 / 32.0  # 1/sqrt(D)


def _build_kernel():
    nc = bacc.Bacc("TRN2", target_bir_lowering=False, debug=False,
                   num_devices=NCORES)

    xt4 = nc.dram_tensor("xt4", [8, P, 8, 256], F32, kind="ExternalInput").ap()
    xq4 = nc.dram_tensor("xq4", [4, P, 8, 256], F32, kind="ExternalInput").ap()
    wq4 = nc.dram_tensor("wq4", [4, P, 8, 256], F32, kind="ExternalInput").ap()
    wk4 = nc.dram_tensor("wk4", [4, P, 8, 256], F32, kind="ExternalInput").ap()
    wv4 = nc.dram_tensor("wv4", [4, P, 8, 256], F32, kind="ExternalInput").ap()
    maskT = nc.dram_tensor("maskT", [P, 4, 256], BF16, kind="ExternalInput").ap()
    out = nc.dram_tensor("out", [NQ, D], F32, kind="ExternalOutput").ap()
    # cores 2b (h=0) and 2b+1 (h=1) of batch b exchange K/V halves
    GROUPS = [[0, 1], [2, 3], [4, 5], [6, 7]]

    with tile.TileContext(nc) as tc, ExitStack() as ctx:
        const = ctx.enter_context(tc.tile_pool(name="const", bufs=1))
        persist = ctx.enter_context(tc.tile_pool(name="persist", bufs=1))

        ident = const.tile([P, P], F32)
        make_identity(nc, ident[:])
        ones = const.tile([P, 1], BF16)
        nc.gpsimd.memset(ones[:], 1.0)
        mask_sb = const.tile([P, 4, 256], BF16)
        nc.sync.dma_start(mask_sb[:], maskT[:])

        QT = persist.tile([P, 8, NQ], BF16)    # [e_in_tile, e_tile, q]
        KT = persist.tile([P, 8, S], BF16)     # [e_in_tile, e_tile, k]
        V = persist.tile([P, 16, D], BF16)     # [k_in_tile, k_tile, e]
        denT = persist.tile([P, 2 * NSLOT], F32)
        rinv = persist.tile([P, 2 * NSLOT], F32)

        # ---------------- projection phases ----------------
        # K (and Q) are projected in full on every core — keeping scores
        # off the collective critical path — while V is projected only for
        # the core's own four 256-row query blocks {2t+h} (so the xq tiles
        # double as the V stationary operand) and pair-wise all-gathered in
        # two stages.  Gather output is rank-ordered: stage s, rank r,
        # piece j holds global block 4s+2j+r on both cores — SPMD-uniform.
        # V-own runs first so the collectives launch early; their latency
        # hides under the K/Q projections.
        with tc.tile_pool(name="wsb", bufs=1) as wsb_pool, \
             tc.tile_pool(name="stage", bufs=2) as stage_pool, \
             tc.tile_pool(name="xc", bufs=6) as xc_pool, \
             tc.tile_pool(name="kvh", bufs=1) as kvh_pool, \
             tc.tile_pool(name="ccdram", bufs=2, space="DRAM") as ccdram, \
             tc.tile_pool(name="pproj", bufs=4, space="PSUM") as pproj:

            def load_cast(dst, dram_chunk):
                st = stage_pool.tile([P, 8, 256], F32, tag="stage")
                nc.sync.dma_start(st[:], dram_chunk)
                nc.vector.tensor_copy(dst, st[:])

            wq_sb = wsb_pool.tile([P, 8, D], BF16, tag="wq")
            wk_sb = wsb_pool.tile([P, 8, D], BF16, tag="wk")
            wv_sb = wsb_pool.tile([P, 8, D], BF16, tag="wv")

            def load_chunk_pair(dram4, i):
                xc = xc_pool.tile([P, 8, 512], BF16, tag="xc")
                for s2 in range(2):
                    load_cast(xc[:, :, 256 * s2:256 * (s2 + 1)],
                              dram4[2 * i + s2])
                return xc

            # Loads are emitted lazily, right before the compute group that
            # consumes them, so the DVE cast/evac FIFO order matches true
            # data-arrival order (no head-of-line blocking).

            def emit_v(s, xc):
                # V for the core's own query blocks of stage s (its blocks
                # 4s+h and 4s+2+h, i.e. xq chunk s) + pair all-gather
                vh = kvh_pool.tile([P, 4, D], BF16, tag="vh")
                for ec in range(2):
                    for k4 in range(4):
                        ps = pproj.tile([P, 512], F32, tag="pp")
                        for dt in range(8):
                            nc.tensor.matmul(
                                ps[:], xc[:, dt, P * k4:P * (k4 + 1)],
                                wv_sb[:, dt, 512 * ec:512 * (ec + 1)],
                                start=(dt == 0), stop=(dt == 7))
                        nc.scalar.copy(vh[:, k4, 512 * ec:512 * (ec + 1)],
                                       ps[:])
                cc_in = ccdram.tile([P, 4096], BF16, tag="cc_in")
                cc_out = ccdram.tile([2, P, 4096], BF16, tag="cc_out")
                # all collective-adjacent DMAs live on the gpsimd queue so
                # they never head-of-line-block the input-load sync queue
                nc.gpsimd.dma_start(cc_in[:],
                                    vh[:].rearrange("p a b -> p (a b)"))
                nc.gpsimd.collective_compute(
                    "AllGather", mybir.AluOpType.bypass,
                    replica_groups=GROUPS,
                    ins=[cc_in[:]], outs=[cc_out[:]])
                # stage s, rank r, piece j -> global 256-row block 4s+2j+r
                for r in range(2):
                    co = cc_out[r].rearrange("p (a b) -> p a b", a=4)
                    for j in range(2):
                        g = 4 * s + 2 * j + r
                        nc.gpsimd.dma_start(V[:, 2 * g:2 * g + 2, :],
                                            co[:, 2 * j:2 * j + 2, :])

            def emit_k(kc, xc):
                for et in range(8):
                    ps = pproj.tile([P, 512], F32, tag="pp")
                    for dt in range(8):
                        nc.tensor.matmul(
                            ps[:], wk_sb[:, dt, P * et:P * (et + 1)],
                            xc[:, dt, :],
                            start=(dt == 0), stop=(dt == 7))
                    nc.scalar.copy(KT[:, et, 512 * kc:512 * (kc + 1)], ps[:])

            def emit_q(qc, xqc):
                for et in range(8):
                    ps = pproj.tile([P, 512], F32, tag="pp")
                    for dt in range(8):
                        nc.tensor.matmul(
                            ps[:], wq_sb[:, dt, P * et:P * (et + 1)],
                            xqc[:, dt, :],
                            start=(dt == 0), stop=(dt == 7))
                    nc.scalar.copy(QT[:, et, 512 * qc:512 * (qc + 1)], ps[:])

            # eager loads, in consumption order; PE order matches arrival
            for c in range(2):
                load_cast(wv_sb[:, :, 256 * c:256 * (c + 1)], wv4[c])
            xq_tiles = [load_chunk_pair(xq4, 0)]
            for c in range(2, 4):
                load_cast(wv_sb[:, :, 256 * c:256 * (c + 1)], wv4[c])
            xq_tiles.append(load_chunk_pair(xq4, 1))
            for c in range(4):
                load_cast(wq_sb[:, :, 256 * c:256 * (c + 1)], wq4[c])
            for c in range(4):
                load_cast(wk_sb[:, :, 256 * c:256 * (c + 1)], wk4[c])
            xg_tiles = [load_chunk_pair(xt4, 0)]

            emit_v(0, xq_tiles[0])
            emit_v(1, xq_tiles[1])
            emit_q(0, xq_tiles[0])
            emit_q(1, xq_tiles[1])
            emit_k(0, xg_tiles[0])
            xg_tiles.append(load_chunk_pair(xt4, 1))
            emit_k(1, xg_tiles[1])
            xg_tiles.append(load_chunk_pair(xt4, 2))
            emit_k(2, xg_tiles[2])
            xg_tiles.append(load_chunk_pair(xt4, 3))
            emit_k(3, xg_tiles[3])

        # ---------------- attention phase ----------------
        with tc.tile_pool(name="ps_s", bufs=2, space="PSUM") as ps_s, \
             tc.tile_pool(name="ps_d", bufs=1, space="PSUM") as ps_d, \
             tc.tile_pool(name="ps_o", bufs=1, space="PSUM") as ps_o, \
             tc.tile_pool(name="ps_t", bufs=1, space="PSUM") as ps_t, \
             tc.tile_pool(name="expp", bufs=4) as expp, \
             tc.tile_pool(name="tmpp", bufs=2) as tmpp, \
             tc.tile_pool(name="obuf", bufs=2) as obufp, \
             tc.tile_pool(name="osb", bufs=4) as osbp, \
             tc.tile_pool(name="dendram", bufs=1, space="DRAM") as dendramp, \
             tc.tile_pool(name="dsb", bufs=2) as dsbp:

            den_dram = dendramp.tile([NSLOT, QB], F32)
            post_queue = deque()

            for t in range(NSLOT):
                nkt = 4 * (t + 1)
                po = ps_o.tile([P, 8, 256], F32, tag="po")
                pd = ps_d.tile([P, 256], F32, tag="pd")
                exp_tiles = {}

                def emit_pv(k, t=t, nkt=nkt, po=po, pd=pd, exp_tiles=None):
                    e = exp_tiles[k]
                    nc.tensor.matmul(pd[0:1, :], ones[:, 0:1], e[:],
                                     start=(k == 0), stop=(k == nkt - 1))
                    # po packs two 256-wide accumulation regions per 2KB psum
                    # bank; only one accumulation group may exist per bank, so
                    # start/stop fire on the first/last matmul touching the
                    # bank (even/odd e-tile respectively).
                    for et in range(8):
                        nc.tensor.matmul(po[:, et, :],
                                         V[:, k, P * et:P * (et + 1)], e[:],
                                         start=(k == 0 and et % 2 == 0),
                                         stop=(k == nkt - 1 and et % 2 == 1))

                for kt in range(nkt):
                    ps = ps_s.tile([P, 256], F32, tag="ps")
                    for et in range(8):
                        nc.tensor.matmul(
                            ps[:], KT[:, et, P * kt:P * (kt + 1)],
                            QT[:, et, QB * t:QB * (t + 1)],
                            start=(et == 0), stop=(et == 7))
                    e = expp.tile([P, 256], BF16, tag="exp")
                    j = kt - 4 * t
                    if j >= 0:
                        tmp = tmpp.tile([P, 256], BF16, tag="tmp")
                        nc.scalar.activation(tmp[:], ps[:],
                                             mybir.ActivationFunctionType.Exp,
                                             scale=SCALE)
                        nc.vector.tensor_tensor(e[:], tmp[:], mask_sb[:, j, :],
                                                mybir.AluOpType.mult)
                    else:
                        nc.scalar.activation(e[:], ps[:],
                                             mybir.ActivationFunctionType.Exp,
                                             scale=SCALE)
                    exp_tiles[kt] = e
                    if kt >= 1:
                        emit_pv(kt - 1, exp_tiles=exp_tiles)
                    if post_queue:
                        post_queue.popleft()()
                emit_pv(nkt - 1, exp_tiles=exp_tiles)

                # denominator -> [q,1] layout via DRAM roundtrip
                dsb = dsbp.tile([1, 256], F32, tag="den")
                nc.vector.tensor_copy(dsb[:], pd[0:1, :])
                nc.sync.dma_start(den_dram[t:t + 1, :], dsb[:])
                nc.sync.dma_start(
                    denT[:, 2 * t:2 * t + 2],
                    den_dram[t:t + 1, :].rearrange("o (c p) -> (o p) c", p=P))
                nc.vector.reciprocal(rinv[:, 2 * t:2 * t + 2],
                                     denT[:, 2 * t:2 * t + 2])

                # evacuate PV accumulator, then queue transpose+normalize work
                ob = obufp.tile([P, 8, 256], F32, tag="ob")
                nc.vector.tensor_copy(ob[:], po[:])

                for half in range(2):
                    osb = osbp.tile([P, D], F32, tag="osb")

                    def mk_tr(et, ob=ob, osb=osb, t=t, half=half):
                        def doit():
                            tr = ps_t.tile([P, P], F32, tag="tr")
                            nc.tensor.transpose(
                                tr[:], ob[:, et, P * half:P * (half + 1)],
                                ident[:])
                            # normalize on the (lightly-loaded) scalar engine
                            nc.scalar.mul(
                                osb[:, P * et:P * (et + 1)], tr[:],
                                rinv[:, 2 * t + half:2 * t + half + 1])
                        return doit

                    for et in range(8):
                        post_queue.append(mk_tr(et))

                    def mk_out(osb=osb, t=t, half=half):
                        def doit():
                            r0 = QB * t + P * half
                            nc.sync.dma_start(out[r0:r0 + P, :], osb[:])
                        return doit

                    post_queue.append(mk_out())

            while post_queue:
                post_queue.popleft()()

    nc.compile()
    return nc


_NC_CACHE = None


def _get_nc():
    global _NC_CACHE
    if _NC_CACHE is None:
        _NC_CACHE = _build_kernel()
    return _NC_CACHE


def _make_masks():
    kk = np.arange(P)[:, None]
    qq = np.arange(256)[None, :]
    diag0 = (qq >= kk).astype(np.float32)
    diag1 = (qq >= kk + P).astype(np.float32)
    m = {}
    for h in range(2):
        mt = np.zeros((P, 4, 256), np.float32)
        if h == 0:
            mt[:, 0], mt[:, 1] = diag0, diag1
        else:
            mt[:, 0], mt[:, 1] = 1.0, 1.0
            mt[:, 2], mt[:, 3] = diag0, diag1
        m[h] = mt.astype(ml_dtypes.bfloat16)
    return m


def _prep_inputs(x, Wq, Wk, Wv):
    def w4(W):
        return np.ascontiguousarray(
            W.reshape(8, P, 4, 256).transpose(2, 1, 0, 3))

    wq4, wk4, wv4 = w4(Wq), w4(Wk), w4(Wv)
    masks = _make_masks()
    in_maps = []
    for core in range(NCORES):
        b, h = divmod(core, 2)
        xb = np.asarray(x[b])
        xt4 = np.ascontiguousarray(
            xb.reshape(8, 256, 8, P).transpose(0, 3, 2, 1))
        order = np.concatenate(
            [np.arange(QB * (2 * t + h), QB * (2 * t + h) + QB)
             for t in range(NSLOT)])
        xq = xb[order]
        xq4 = np.ascontiguousarray(
            xq.reshape(4, 256, 8, P).transpose(0, 3, 2, 1))
        in_maps.append({
            "xt4": xt4, "xq4": xq4,
            "wq4": wq4, "wk4": wk4, "wv4": wv4,
            "maskT": masks[h],
        })
    return in_maps


def run(inputs, trace=False):
    nc = _get_nc()
    in_maps = _prep_inputs(inputs["x"], inputs["Wq"], inputs["Wk"],
                           inputs["Wv"])
    res = bass_utils.run_bass_kernel_spmd(
        nc, in_maps, core_ids=list(range(NCORES)), trace=trace)
    out = np.empty((B, S, D), np.float32)
    for core in range(NCORES):
        b, h = divmod(core, 2)
        oc = res.results[core]["out"]
        for t in range(NSLOT):
            out[b, QB * (2 * t + h):QB * (2 * t + h) + QB] = \
                oc[QB * t:QB * t + QB]
    return out, res


def kernel(**inputs):
    out, _ = run(inputs, trace=False)
    return out
